# revision 27
# baseline (speedup 1.0000x reference)
"""DLinear fused kernel for 8 TRN2 NeuronCores.

Math: the whole module is linear in x.
  trend = x @ A^T (A = edge-padded moving-average matrix, window 25)
  out[b,n,p] = sum_c wf_c * ( x[b,c,n,:] @ Weff[p,:] ) + bias[p]
  Weff = Ws + (Wt-Ws)@A,  bias = sum(wf) * (bs + bt) + bf

Host precomputes (weights in f64, data in f32): Weff fold, channel
combine y = sum_c wf_c x_c, bf16 cast + transpose to [l, bn] layout,
4096 rows per core; bias added on host after the device matmul.

Device per core: pure GEMM out[4096, 336] = y[4096,512] @ Weff.T (bf16).
y-stationary: stationary = y[128l, 128bn], moving = Weff.T chunk
[128l, 336p] -> psum [128bn, 336p], accumulated over 4 l-chunks.
128 LDW+MM pairs x 336 moving cols = 43008 PE cycles (100% PE util).

Schedule (all timings drive the shape):
  - 10 row-blocks of [1,4,4,4,4,4,4,4,2,1]x128 rows. Small first block
    -> first matmul starts ~1.5 us after the DMA ring opens; small last
    block -> the final drain+store tail (which gates the fixed framework
    teardown) is short.
  - all y DMAs in order on the sync HWDGE ring (FIFO => block 0 gets the
    full HBM bandwidth, no packet-interleaving with later blocks);
    W on the scalar ring concurrently; outputs on the scalar ring.
  - every block gets its own SBUF tile (no pool rotation -> no refill
    stalls); ~5 MB of 24 MB SBUF.
  - junk warmup matmuls (memset operands) keep the PE busy while the
    first DMA is in flight so the HAM clock gate opens (1.2->2.4 GHz)
    before the real matmul stream.
  - psum drains (f32->bf16) all on the otherwise-idle VectorE.
"""

import numpy as np
import ml_dtypes

import concourse.bacc as bacc
import concourse.mybir as mybir
import concourse.tile as tile
from concourse.bass_utils import run_bass_kernel_spmd

N_CORES = 8
B, C, N, L, P = 64, 3, 512, 512, 336
KERNEL_W, PAD = 25, 12
BPC = B // N_CORES          # batches per core = 8
BN = BPC * N                # rows per core = 4096
NU = BN // 128              # 128-row units per core = 32
LC = 4                      # l chunks of 128
BLOCKS = [1, 2, 3, 4, 4, 4, 4, 4, 3, 2, 1]   # units per block (sum = 32)
# 34 x ~107ns cold N=128 junk matmuls = ~3.6us of continuous PE activity:
# enough to flip the HAM clock gate (needs >=3.4us of gapless nonzero-data
# activity; 30 is NOT enough) while the first input DMAs are in flight,
# so the real matmul stream runs warm (2.4 GHz) from the start.
N_WARM = 34

BF16 = mybir.dt.bfloat16
F32 = mybir.dt.float32

LAST_RESULT = None
_CACHE = {}


def _movavg_matrix():
    A = np.zeros((L, L), np.float64)
    for lp in range(L):
        for kk in range(lp - PAD, lp + PAD + 1):
            A[lp, min(max(kk, 0), L - 1)] += 1.0 / KERNEL_W
    return A


def _build():
    assert sum(BLOCKS) == NU
    nc = bacc.Bacc("TRN2", target_bir_lowering=False, debug=False)
    # y: [part][u][k][col] -- per-partition contiguous KBs per unit
    y_d = nc.dram_tensor("y", (128, NU, LC, 128), BF16, kind="ExternalInput")
    w_d = nc.dram_tensor("w", (128, LC, P), BF16, kind="ExternalInput")
    o_d = nc.dram_tensor("o", (128, NU, P), BF16, kind="ExternalOutput")

    with tile.TileContext(nc) as tc:
        with (
            tc.tile_pool(name="const", bufs=1) as constp,
            tc.tile_pool(name="warm", bufs=1) as warmp,
            tc.tile_pool(name="yin", bufs=1) as yinp,
            tc.tile_pool(name="ps", bufs=6, space="PSUM") as psp,
            tc.tile_pool(name="pswarm", bufs=1, space="PSUM") as pswp,
            tc.tile_pool(name="ostage", bufs=1) as osp,
        ):
            # warmup matmuls: no DMA dependency. Varying nonzero junk data
            # (iota) so the PE activity monitor actually sees datapath
            # activity -- all-zero matmuls don't un-throttle the clock.
            wst = warmp.tile([128, 128], BF16, tag="wst", name="wst")
            nc.gpsimd.iota(wst[:], [[1, 128]], channel_multiplier=3,
                           allow_small_or_imprecise_dtypes=True)
            psw = pswp.tile([128, 128], F32, tag="psw", name="psw")
            for i in range(N_WARM):
                nc.tensor.matmul(psw[:], wst[:], wst[:], start=True, stop=True)

            # W on the gpsimd SWDGE ring: keeps both HWDGE rings clear of
            # weight traffic while y0/y1 pay their completion receipts
            # (SWDGE delivers 344 KiB by ~10.9us, before the first matmul).
            wt = constp.tile([128, LC * P], BF16, tag="w", name="w")
            nc.gpsimd.dma_start(wt[:], w_d[:])
            wts = [wt[:, k * P:(k + 1) * P] for k in range(LC)]

            u0 = 0
            for b, m in enumerate(BLOCKS):
                yt = yinp.tile([128, m * LC * 128], BF16, tag=f"y{b}",
                               name=f"y{b}")
                nc.sync.dma_start(yt[:], y_d[:, u0:u0 + m])
                ost = osp.tile([128, m * P], BF16, tag=f"ost{b}",
                               name=f"ost{b}")
                for j in range(m):
                    ps = psp.tile([128, P], F32, tag="ps", name=f"ps{b}_{j}")
                    for k in range(LC):
                        nc.tensor.matmul(
                            ps[:],
                            yt[:, (j * LC + k) * 128:(j * LC + k + 1) * 128],
                            wts[k],
                            start=(k == 0),
                            stop=(k == LC - 1),
                        )
                    nc.vector.tensor_copy(ost[:, j * P:(j + 1) * P], ps[:])
                # last block's store on the (by then idle) sync ring
                if b == len(BLOCKS) - 1:
                    nc.sync.dma_start(o_d[:, u0:u0 + m], ost[:])
                else:
                    nc.scalar.dma_start(o_d[:, u0:u0 + m], ost[:])
                u0 += m

    nc.compile()
    return nc


def kernel(x, Ws, bs, Wt, bt, Wf, bf):
    global LAST_RESULT
    # ---- host-side weight folding (f64, weights only) ----
    A = _movavg_matrix()
    Weff = Ws.astype(np.float64) + (Wt.astype(np.float64) - Ws.astype(np.float64)) @ A
    WT = np.ascontiguousarray(
        Weff.T.reshape(LC, 128, P).transpose(1, 0, 2)
    ).astype(ml_dtypes.bfloat16)                       # (128, LC, P)
    wf = Wf[0].astype(np.float64)                      # (3,)
    bias = (wf.sum() * (bs.astype(np.float64) + bt.astype(np.float64))
            + float(bf[0])).astype(np.float32)         # (336,)

    if "nc" not in _CACHE:
        _CACHE["nc"] = _build()
    nc = _CACHE["nc"]

    # ---- host-side channel combine + sharding / layout ----
    xf = x.astype(np.float32, copy=False)
    y = (np.float32(wf[0]) * xf[:, 0]
         + np.float32(wf[1]) * xf[:, 1]
         + np.float32(wf[2]) * xf[:, 2])               # (64, 512, 512)
    yb = y.reshape(N_CORES, BN, L).astype(ml_dtypes.bfloat16)

    in_maps = []
    for i in range(N_CORES):
        yT = yb[i].T                                    # (512 l, 4096 bn)
        # [part][u][k][col]: l = k*128+part, bn = u*128+col
        yT = yT.reshape(LC, 128, NU, 128).transpose(1, 2, 0, 3)
        in_maps.append({
            "y": np.ascontiguousarray(yT),              # (128, NU, LC, 128)
            "w": WT,
        })

    res = run_bass_kernel_spmd(nc, in_maps, core_ids=list(range(N_CORES)))
    LAST_RESULT = res

    # ---- gather / unshard ----
    outs = []
    for i in range(N_CORES):
        o = res.results[i]["o"].astype(np.float32)      # (128, NU, P)
        o = o.transpose(1, 0, 2).reshape(BN, P)         # rows bn = u*128+part
        outs.append(o)
    out = np.stack(outs).reshape(B, N, P) + bias        # (64, 512, 336)
    return out[:, None].astype(np.float32)


# revision 28
# speedup vs baseline: 1.0039x; 1.0039x over previous
"""DLinear fused kernel for 8 TRN2 NeuronCores.

Math: the whole module is linear in x.
  trend = x @ A^T (A = edge-padded moving-average matrix, window 25)
  out[b,n,p] = sum_c wf_c * ( x[b,c,n,:] @ Weff[p,:] ) + bias[p]
  Weff = Ws + (Wt-Ws)@A,  bias = sum(wf) * (bs + bt) + bf

Host precomputes (weights in f64, data in f32): Weff fold, channel
combine y = sum_c wf_c x_c, bf16 cast + transpose to [l, bn] layout,
4096 rows per core; bias added on host after the device matmul.

Device per core: pure GEMM out[4096, 336] = y[4096,512] @ Weff.T (bf16).
y-stationary: stationary = y[128l, 128bn], moving = Weff.T chunk
[128l, 336p] -> psum [128bn, 336p], accumulated over 4 l-chunks.
128 LDW+MM pairs x 336 moving cols = 43008 PE cycles (100% PE util).

Schedule (all timings drive the shape):
  - 10 row-blocks of [1,4,4,4,4,4,4,4,2,1]x128 rows. Small first block
    -> first matmul starts ~1.5 us after the DMA ring opens; small last
    block -> the final drain+store tail (which gates the fixed framework
    teardown) is short.
  - all y DMAs in order on the sync HWDGE ring (FIFO => block 0 gets the
    full HBM bandwidth, no packet-interleaving with later blocks);
    W on the scalar ring concurrently; outputs on the scalar ring.
  - every block gets its own SBUF tile (no pool rotation -> no refill
    stalls); ~5 MB of 24 MB SBUF.
  - junk warmup matmuls (memset operands) keep the PE busy while the
    first DMA is in flight so the HAM clock gate opens (1.2->2.4 GHz)
    before the real matmul stream.
  - psum drains (f32->bf16) all on the otherwise-idle VectorE.
"""

import numpy as np
import ml_dtypes

import concourse.bacc as bacc
import concourse.mybir as mybir
import concourse.tile as tile
from concourse.bass_utils import run_bass_kernel_spmd

N_CORES = 8
B, C, N, L, P = 64, 3, 512, 512, 336
KERNEL_W, PAD = 25, 12
BPC = B // N_CORES          # batches per core = 8
BN = BPC * N                # rows per core = 4096
NU = BN // 128              # 128-row units per core = 32
LC = 4                      # l chunks of 128
BLOCKS = [1, 2, 3, 4, 4, 4, 4, 4, 3, 2, 1]   # units per block (sum = 32)
# 34 x ~107ns cold N=128 junk matmuls = ~3.6us of continuous PE activity:
# enough to flip the HAM clock gate (needs >=3.4us of gapless nonzero-data
# activity; 30 is NOT enough) while the first input DMAs are in flight,
# so the real matmul stream runs warm (2.4 GHz) from the start.
N_WARM = 34

BF16 = mybir.dt.bfloat16
F32 = mybir.dt.float32

LAST_RESULT = None
_CACHE = {}


def _movavg_matrix():
    A = np.zeros((L, L), np.float64)
    for lp in range(L):
        for kk in range(lp - PAD, lp + PAD + 1):
            A[lp, min(max(kk, 0), L - 1)] += 1.0 / KERNEL_W
    return A


def _build():
    assert sum(BLOCKS) == NU
    nc = bacc.Bacc("TRN2", target_bir_lowering=False, debug=False)
    # y: [part][u][k][col] -- per-partition contiguous KBs per unit
    y_d = nc.dram_tensor("y", (128, NU, LC, 128), BF16, kind="ExternalInput")
    w_d = nc.dram_tensor("w", (128, LC, P), BF16, kind="ExternalInput")
    o_d = nc.dram_tensor("o", (128, NU, P), BF16, kind="ExternalOutput")

    with tile.TileContext(nc) as tc:
        with (
            tc.tile_pool(name="const", bufs=1) as constp,
            tc.tile_pool(name="warm", bufs=1) as warmp,
            tc.tile_pool(name="yin", bufs=1) as yinp,
            tc.tile_pool(name="ps", bufs=6, space="PSUM") as psp,
            tc.tile_pool(name="pswarm", bufs=1, space="PSUM") as pswp,
            tc.tile_pool(name="ostage", bufs=1) as osp,
        ):
            # warmup matmuls: no DMA dependency. Varying nonzero junk data
            # (iota) so the PE activity monitor actually sees datapath
            # activity -- all-zero matmuls don't un-throttle the clock.
            wst = warmp.tile([128, 128], BF16, tag="wst", name="wst")
            nc.gpsimd.iota(wst[:], [[1, 128]], channel_multiplier=3,
                           allow_small_or_imprecise_dtypes=True)
            psw = pswp.tile([128, 128], F32, tag="psw", name="psw")
            for i in range(N_WARM):
                nc.tensor.matmul(psw[:], wst[:], wst[:], start=True, stop=True)

            # W in two chunks: the first matmuls only wait for k=0,1
            wts = []
            for h in range(2):
                wh = constp.tile([128, 2 * P], BF16, tag=f"w{h}", name=f"w{h}")
                nc.scalar.dma_start(wh[:], w_d[:, 2 * h:2 * h + 2])
                wts.extend([wh[:, 0:P], wh[:, P:2 * P]])

            u0 = 0
            for b, m in enumerate(BLOCKS):
                yt = yinp.tile([128, m * LC * 128], BF16, tag=f"y{b}",
                               name=f"y{b}")
                nc.sync.dma_start(yt[:], y_d[:, u0:u0 + m])
                ost = osp.tile([128, m * P], BF16, tag=f"ost{b}",
                               name=f"ost{b}")
                for j in range(m):
                    ps = psp.tile([128, P], F32, tag="ps", name=f"ps{b}_{j}")
                    for k in range(LC):
                        nc.tensor.matmul(
                            ps[:],
                            yt[:, (j * LC + k) * 128:(j * LC + k + 1) * 128],
                            wts[k],
                            start=(k == 0),
                            stop=(k == LC - 1),
                        )
                    nc.vector.tensor_copy(ost[:, j * P:(j + 1) * P], ps[:])
                # last block's store on the (by then idle) sync ring
                if b == len(BLOCKS) - 1:
                    nc.sync.dma_start(o_d[:, u0:u0 + m], ost[:])
                else:
                    nc.scalar.dma_start(o_d[:, u0:u0 + m], ost[:])
                u0 += m

    nc.compile()
    return nc


def kernel(x, Ws, bs, Wt, bt, Wf, bf):
    global LAST_RESULT
    # ---- host-side weight folding (f64, weights only) ----
    A = _movavg_matrix()
    Weff = Ws.astype(np.float64) + (Wt.astype(np.float64) - Ws.astype(np.float64)) @ A
    WT = np.ascontiguousarray(
        Weff.T.reshape(LC, 128, P).transpose(1, 0, 2)
    ).astype(ml_dtypes.bfloat16)                       # (128, LC, P)
    wf = Wf[0].astype(np.float64)                      # (3,)
    bias = (wf.sum() * (bs.astype(np.float64) + bt.astype(np.float64))
            + float(bf[0])).astype(np.float32)         # (336,)

    if "nc" not in _CACHE:
        _CACHE["nc"] = _build()
    nc = _CACHE["nc"]

    # ---- host-side channel combine + sharding / layout ----
    xf = x.astype(np.float32, copy=False)
    y = (np.float32(wf[0]) * xf[:, 0]
         + np.float32(wf[1]) * xf[:, 1]
         + np.float32(wf[2]) * xf[:, 2])               # (64, 512, 512)
    yb = y.reshape(N_CORES, BN, L).astype(ml_dtypes.bfloat16)

    in_maps = []
    for i in range(N_CORES):
        yT = yb[i].T                                    # (512 l, 4096 bn)
        # [part][u][k][col]: l = k*128+part, bn = u*128+col
        yT = yT.reshape(LC, 128, NU, 128).transpose(1, 2, 0, 3)
        in_maps.append({
            "y": np.ascontiguousarray(yT),              # (128, NU, LC, 128)
            "w": WT,
        })

    res = run_bass_kernel_spmd(nc, in_maps, core_ids=list(range(N_CORES)))
    LAST_RESULT = res

    # ---- gather / unshard ----
    outs = []
    for i in range(N_CORES):
        o = res.results[i]["o"].astype(np.float32)      # (128, NU, P)
        o = o.transpose(1, 0, 2).reshape(BN, P)         # rows bn = u*128+part
        outs.append(o)
    out = np.stack(outs).reshape(B, N, P) + bias        # (64, 512, 336)
    return out[:, None].astype(np.float32)
